# revision 25
# baseline (speedup 1.0000x reference)
"""Trainium2 Bass kernel for CachedHeavyRecentAttentionMasker.

Pipeline (8 NeuronCores, SPMD):
  Phase A (q-sharded): per-core softmax-importance partials.
      importance[h,k] = sum_q exp(x[h,q,k]) / D[h,q],  D = row sums of exp.
      ACT computes exp (f32) with accum_out row sums; DVE casts to bf16;
      PE reduces over q via bf16 matmul with lhsT = bf16(1/D).
  Host: exact top-204 per head (device importance + f64 re-resolution of
      boundary-band candidates), union over groups of 7, exact density.
  Phase C (head-sharded): per-core mask tiles assembled from constant
      SBUF patterns: out = min(max(heavy_row, band_master), causal_master)
      computed only on the 332-wide diagonal band; pure DMA elsewhere.
"""
import os

import numpy as np

import concourse.bacc as bacc
import concourse.mybir as mybir
from concourse.tile import TileContext
from concourse.bass_utils import run_bass_kernel_spmd

# Problem constants (hardcoded per harness contract)
BS = 1
H = 28            # heads
GS = 7            # group size
G = H // GS       # 4 groups
Q = 2048
K = 2048
HB = 204          # heavy budget = int(0.1 * K) (>=1)
RB = 204          # recent budget = int(0.1 * K)
NC = 8            # cores
QS = Q // NC      # 256 q rows per core in phase A
P = 128
MIN = float(np.finfo(np.float32).min)
F32 = mybir.dt.float32
BF16 = mybir.dt.bfloat16
MARGIN = 3e-3     # candidate band half-width around the top-204 boundary

_CACHE = {}
LAST_TIMES = {}


def _build_phase_a():
    """Per-core: x [H*QS, K] f32 -> imp [H, K] f32 (partial), d [H*QS, 1] f32."""
    nc = bacc.Bacc("TRN2")
    x_d = nc.declare_dram_parameter("x", [H * QS, K], F32, isOutput=False)
    imp_d = nc.declare_dram_parameter("imp", [H, K], F32, isOutput=True)
    d_d = nc.declare_dram_parameter("d", [H * QS, 1], F32, isOutput=True)
    nt = QS // P  # 2 q-tiles per head per core

    with TileContext(nc) as tc:
        with tc.tile_pool(name="px", bufs=9) as xpool, \
             tc.tile_pool(name="pe_", bufs=8) as epool, \
             tc.tile_pool(name="pb", bufs=8) as bpool, \
             tc.tile_pool(name="sm", bufs=12) as spool, \
             tc.tile_pool(name="im", bufs=2) as ipool, \
             tc.tile_pool(name="ps", bufs=2, space="PSUM") as psum:
            # 4 heads per PSUM tile, at partition offsets {0,32,64,96}.
            # Copy-out of quad g is emitted mid-quad g+1 so the in-order
            # DVE stream doesn't stall the next quad's per-tile work.
            pending = []

            def flush_pending():
                pg, pt = pending.pop(0)
                impt = ipool.tile([P, K], F32, tag="impt")
                nc.vector.tensor_copy(impt[:97, :], pt[0:97, :])
                nc.scalar.dma_start(imp_d[4 * pg:4 * pg + 4, :],
                                    impt[0:128:32, :])

            for g in range(H // 4):
                pimp = psum.tile([P, K], F32, tag="pimp")
                for s in range(4):
                    h = 4 * g + s
                    bp = 32 * s
                    for t in range(nt):
                        row = h * QS + t * P
                        xt = xpool.tile([P, K], F32, tag="x")
                        nc.sync.dma_start(xt[:], x_d[row:row + P, :])
                        et = epool.tile([P, K], F32, tag="e")
                        dt_ = spool.tile([P, 1], F32, tag="d")
                        nc.scalar.activation(et[:], xt[:],
                                             mybir.ActivationFunctionType.Exp,
                                             accum_out=dt_[:])
                        rt = spool.tile([P, 1], F32, tag="r")
                        nc.vector.reciprocal(rt[:], dt_[:])
                        rb = spool.tile([P, 1], BF16, tag="rb")
                        nc.vector.tensor_copy(rb[:], rt[:])
                        eb = bpool.tile([P, K], BF16, tag="eb")
                        nc.vector.tensor_copy(eb[:], et[:])
                        for j in range(K // 512):
                            nc.tensor.matmul(
                                pimp[bp:bp + 1, j * 512:(j + 1) * 512],
                                rb[:], eb[:, j * 512:(j + 1) * 512],
                                start=(t == 0), stop=(t == nt - 1),
                                tile_position=(0, bp))
                        nc.scalar.dma_start(d_d[row:row + P, :], dt_[:])
                        if pending and s == 1 and t == 1:
                            flush_pending()
                pending.append((g, pimp))
            while pending:
                flush_pending()
    nc.compile()
    return nc


def _build_phase_c():
    """Per-core: hg [4, K] f32 (0/MIN rows) -> mask [4*Q, K] f32."""
    nc = bacc.Bacc("TRN2")
    hg_d = nc.declare_dram_parameter("hg", [4, K], F32, isOutput=False)
    bm_d = nc.declare_dram_parameter("bm", [P, RB + P], F32, isOutput=False)
    cm_d = nc.declare_dram_parameter("cm", [P, RB + P], F32, isOutput=False)
    out_d = nc.declare_dram_parameter("mask", [4 * Q, K], F32, isOutput=True)
    W = RB + P  # 332

    with TileContext(nc) as tc:
        with tc.tile_pool(name="cs", bufs=1) as cpool, \
             tc.tile_pool(name="cb", bufs=6) as bpool:
            hgt = []
            for s in range(4):
                hgrow = cpool.tile([1, K], F32, tag=f"hgr{s}")
                nc.sync.dma_start(hgrow[:], hg_d[s:s + 1, :])
                t_ = cpool.tile([P, K], F32, tag=f"hg{s}")
                nc.gpsimd.partition_broadcast(t_[:], hgrow[:])
                hgt.append(t_)
            bmt = cpool.tile([P, W], F32, tag="bm")
            cmt = cpool.tile([P, W], F32, tag="cm")
            nc.sync.dma_start(bmt[:], bm_d[:])
            nc.sync.dma_start(cmt[:], cm_d[:])
            mint = cpool.tile([P, Q - P], F32, tag="mint")
            nc.any.memset(mint[:], MIN)

            for s in range(4):
                for t in range(Q // P):
                    q0 = t * P
                    s0 = max(0, q0 - RB)
                    w = q0 + P - s0
                    j0 = W - w
                    row = s * Q + q0
                    cb = bpool.tile([P, W], F32, tag="cb")
                    nc.vector.tensor_tensor(cb[:, :w], hgt[s][:, s0:s0 + w],
                                            bmt[:, j0:j0 + w],
                                            op=mybir.AluOpType.max)
                    nc.vector.tensor_tensor(cb[:, :w], cb[:, :w],
                                            cmt[:, j0:j0 + w],
                                            op=mybir.AluOpType.min)
                    if s0 > 0:
                        nc.sync.dma_start(out_d[row:row + P, 0:s0],
                                          hgt[s][:, 0:s0])
                    nc.scalar.dma_start(out_d[row:row + P, s0:s0 + w], cb[:, :w])
                    if q0 + P < K:
                        nc.sync.dma_start(out_d[row:row + P, q0 + P:K],
                                          mint[:, 0:K - (q0 + P)])
    nc.compile()
    return nc


def _masters():
    pp = np.arange(P)[:, None]
    jj = np.arange(RB + P)[None, :]
    bm = np.where((jj >= pp) & (jj <= pp + RB), 0.0, MIN).astype(np.float32)
    cm = np.where(jj <= RB + pp, 0.0, MIN).astype(np.float32)
    return bm, cm


def _select_topk(imp_dev, d_dev, aw):
    """Exact per-head top-HB sets: device importance + f64 band resolution."""
    heavy = np.zeros((H, K), dtype=bool)
    for h in range(H):
        v = imp_dev[h]
        order = np.argsort(-v, kind="stable")
        thr = v[order[HB - 1]]
        hi, lo = thr + MARGIN, thr - MARGIN
        certain = v > hi
        cand = np.flatnonzero((v >= lo) & (v <= hi))
        need = HB - int(certain.sum())
        if need < 0 or need > cand.size:
            # margin assumption violated; fall back to full f64 recompute
            cand = np.arange(K)
            certain = np.zeros(K, dtype=bool)
            need = HB
        heavy[h, certain] = True
        if need > 0:
            x_cols = aw[0, h, :, cand].astype(np.float64)  # [ncand, Q]
            e = np.exp(x_cols)
            vals = (e / d_dev[h][None, :]).sum(axis=1)
            pick = cand[np.argsort(-vals, kind="stable")[:need]]
            heavy[h, pick] = True
    return heavy


def _density(heavy_group):
    q = np.arange(Q)[:, None]
    k = np.arange(K)[None, :]
    causal = k <= q
    recent = causal & (k >= q - RB)
    total = 0
    for g in range(G):
        keep = (heavy_group[g][None, :] & causal) | recent
        total += GS * int(keep.sum())
    dens = total / (BS * H) / (Q * (Q + 1) / 2)
    return np.float32(dens)


def kernel(attn_weights, group_size):
    aw = np.asarray(attn_weights)
    assert aw.shape == (BS, H, Q, K) and aw.dtype == np.float32
    assert int(np.asarray(group_size)) == GS

    if "A" not in _CACHE:
        _CACHE["A"] = _build_phase_a()
    if "C" not in _CACHE:
        _CACHE["C"] = _build_phase_c()
    ncA, ncC = _CACHE["A"], _CACHE["C"]

    trace = bool(int(os.environ.get("KERNEL_TRACE", "0")))
    core_ids = list(range(NC))
    in_maps_a = []
    for c in range(NC):
        xc = np.ascontiguousarray(
            aw[0, :, c * QS:(c + 1) * QS, :]).reshape(H * QS, K)
        in_maps_a.append({"x": xc})
    res_a = run_bass_kernel_spmd(ncA, in_maps_a, core_ids, trace=trace)
    LAST_TIMES["A"] = res_a.exec_time_ns

    imp_dev = np.zeros((H, K), dtype=np.float64)
    d_dev = np.zeros((H, Q), dtype=np.float64)
    for c in range(NC):
        imp_dev += res_a.results[c]["imp"].astype(np.float64)
        d_dev[:, c * QS:(c + 1) * QS] = \
            res_a.results[c]["d"].reshape(H, QS).astype(np.float64)

    heavy = _select_topk(imp_dev, d_dev, aw)
    heavy_group = heavy.reshape(G, GS, K).any(axis=1)  # [G, K]
    hg_rows = np.where(heavy_group, 0.0, MIN).astype(np.float32)  # [G, K]

    bm, cm = _masters()
    in_maps_c = []
    for c in range(NC):
        hgc = np.zeros((4, K), dtype=np.float32)
        for s in range(4):
            h = c * 4 + s
            if h < H:
                hgc[s] = hg_rows[h // GS]
        in_maps_c.append({"hg": hgc, "bm": bm, "cm": cm})
    res_c = run_bass_kernel_spmd(ncC, in_maps_c, core_ids, trace=trace)
    LAST_TIMES["C"] = res_c.exec_time_ns

    fetch_mask = np.empty((BS, H, Q, K), dtype=np.float32)
    for h in range(H):
        c, s = h // 4, h % 4
        fetch_mask[0, h] = \
            res_c.results[c]["mask"][s * Q:(s + 1) * Q, :]

    return fetch_mask, _density(heavy_group)
